# revision 23
# baseline (speedup 1.0000x reference)
"""Fused ViT transformer block on Trainium2, data-parallel over batch across 8 cores.

v2: all-bf16 matmuls (fp32 PSUM accumulate), S^T-layout attention.

Per-core kernel (batch shard [4, 577, 1024]) computes the whole block:
  x + Attn(LN1(x)) -> x2 ; x2 + MLP(LN2(x2))
with all activations resident in SBUF.

Key layout choices:
  - LayerNorms run token-major [t, D] (bn_stats free-dim reductions), write a
    bf16 normalized copy, PE-transpose it into feature-major xnT [D, t] that
    feeds every matmul with the contraction dim on partitions. The raw x tiles
    stay resident in fp32 and serve as the residual stream (proj_b added
    on-chip via a broadcast tile).
  - Attention per head computes S^T[k, q] directly (lhsT = kT k-tile, rhs =
    qT), exp on ACT into bf16 expT, and AV as [V | 1]^T @ expT: the appended
    ones column makes PSUM row 64 the softmax denominators for free. A K=1
    ones-matmul broadcasts 1/denom across partitions and a single DVE multiply
    writes the normalized head output into aT (partition-shifted write for odd
    heads). No post-softmax transpose at all.
  - proj/fc2 are token-major (tokens on PSUM partitions, weights moving), fc1
    feature-major (tokens moving); MLP runs in 1024-wide hidden blocks with an
    fp32 SBUF accumulator initialized to x2 + fc2_bias.

Host-side precompute folds LN affine params into the adjacent weights
(w' = g[:,None]*w, c = b@w + bias); weights are shipped bf16.
"""

import numpy as np
from contextlib import ExitStack

import concourse.bass as bass
import concourse.mybir as mybir
import concourse.tile as tile
from concourse import bacc
from concourse.masks import make_identity

P = 128
F32 = mybir.dt.float32
BF = mybir.dt.bfloat16
AF = mybir.ActivationFunctionType
ALU = mybir.AluOpType


def _chunks(total, step):
    return [(i, min(step, total - i)) for i in range(0, total, step)]


# CoreSim lacks a Gelu implementation; vtest.py overrides this to Tanh.
_ACT_MLP = AF.Gelu


def build_nc(B=4, T=577, D=1024, H=16, HD=64, HID=4096, eps=1e-6):
    """Build the single-core Bass program for a [B, T, D] shard."""
    assert D % P == 0 and H % 2 == 0 and HD == 64 and HID % 1024 == 0
    KD = D // P              # k-tiles over model dim (8)
    NPAIR = H // 2           # head pairs (8)
    NHT = HID // P           # fc1 output tiles (32)
    HB = 1024                # hidden block
    NHB = HID // HB          # 4
    KH = HB // P             # 8
    TOK = B * T
    scale = 1.0 / float(np.sqrt(HD))

    t_tiles = _chunks(T, P)          # [(0,128)x4, (512,65)] token tiles
    NTT = len(t_tiles)
    TB = _chunks(T, 512)             # token col blocks (N axis)
    d_blocks = _chunks(D, 512)

    nc = bacc.Bacc(None, target_bir_lowering=False, debug=False)

    x_d = nc.dram_tensor("x", [TOK, D], F32, kind="ExternalInput")
    wqkv_d = nc.dram_tensor("wqkv", [D, NPAIR, 3 * P], BF, kind="ExternalInput")
    cq_d = nc.dram_tensor("cq", [P, NPAIR * 3], F32, kind="ExternalInput")
    wp_d = nc.dram_tensor("wproj", [D, D], BF, kind="ExternalInput")
    pb_d = nc.dram_tensor("pb", [D], F32, kind="ExternalInput")
    wf1_d = nc.dram_tensor("wfc1", [D, HID], BF, kind="ExternalInput")
    cf1_d = nc.dram_tensor("cf1", [P, NHT], F32, kind="ExternalInput")
    wf2_d = nc.dram_tensor("wfc2", [HID, D], BF, kind="ExternalInput")
    cf2_d = nc.dram_tensor("cf2", [D], F32, kind="ExternalInput")
    out_d = nc.dram_tensor("out", [TOK, D], F32, kind="ExternalOutput")

    with tile.TileContext(nc) as tc, ExitStack() as ctx:
        const = ctx.enter_context(tc.tile_pool(name="const", bufs=1))
        xpool = ctx.enter_context(tc.tile_pool(name="xin", bufs=7))
        xn_p = ctx.enter_context(tc.tile_pool(name="xn", bufs=5))
        xn2_p = ctx.enter_context(tc.tile_pool(name="xn2", bufs=2))
        statp = ctx.enter_context(tc.tile_pool(name="stat", bufs=8))
        rowp = ctx.enter_context(tc.tile_pool(name="rows", bufs=5))
        bcs_p = ctx.enter_context(tc.tile_pool(name="bcs", bufs=2))
        xn1T_p = ctx.enter_context(tc.tile_pool(name="xn1T", bufs=2))
        wq_p = ctx.enter_context(tc.tile_pool(name="wq", bufs=12))
        qkvt_p = ctx.enter_context(tc.tile_pool(name="qkvt", bufs=2))
        vkd_p = ctx.enter_context(tc.tile_pool(name="vkd", bufs=2))
        expT_p = ctx.enter_context(tc.tile_pool(name="expT", bufs=2))
        aT_p = ctx.enter_context(tc.tile_pool(name="aT", bufs=1))
        w5_p = ctx.enter_context(tc.tile_pool(name="w5", bufs=10))
        xn2T_p = ctx.enter_context(tc.tile_pool(name="xn2T", bufs=2))
        hT_p = ctx.enter_context(tc.tile_pool(name="hT", bufs=2))
        oacc_p = ctx.enter_context(tc.tile_pool(name="oacc", bufs=1))
        psA = ctx.enter_context(tc.tile_pool(name="psA", bufs=3, space="PSUM"))
        psAV = ctx.enter_context(tc.tile_pool(name="psAV", bufs=2, space="PSUM"))
        psM = ctx.enter_context(tc.tile_pool(name="psM", bufs=3, space="PSUM"))

        def tr_psum():
            """bf16 [P, 8, 128] transpose target aliased onto an s-ring tile."""
            t = psA.tile([P, 512], F32, tag="s")
            return t.bitcast(BF).rearrange("p (k f) -> p k f", k=8)

        ident = const.tile([P, P], F32)
        make_identity(nc, ident)
        identA = const.tile([P, P], BF)
        nc.vector.tensor_copy(out=identA[:, :], in_=ident[:, :])
        eps_t = const.tile([P, 1], F32)
        nc.vector.memset(eps_t, eps)
        ones1 = const.tile([1, 64], BF)
        nc.vector.memset(ones1, 1.0)
        cq_sb = const.tile([P, NPAIR * 3], F32)
        nc.sync.dma_start(out=cq_sb[:, :], in_=cq_d[:, :])
        cf1_sb = const.tile([P, NHT], F32)
        nc.sync.dma_start(out=cf1_sb[:, :], in_=cf1_d[:, :])

        def bcast_row(dram_vec, tag):
            t = const.tile([P, D], F32, tag=tag)
            ap = dram_vec[:]
            nc.sync.dma_start(
                out=t[:, :],
                in_=bass.AP(tensor=ap.tensor, offset=ap.offset,
                            ap=[[0, P]] + list(ap.ap)),
            )
            return t

        pb_rep = bcast_row(pb_d, "pbrep")
        cf2_rep = bcast_row(cf2_d, "cf2rep")

        def ln_normalize(xts, pool, tag):
            """bn_stats/aggr + batched sqrt/recip + normalized bf16 copies.
            DVE/ACT only — no PE instructions."""
            mvs = statp.tile([P, NTT, 2], F32, tag="mv" + tag)
            nc.vector.memset(mvs, 1.0)
            for ti, (xt, t0, tsz) in enumerate(xts):
                nsub = max(1, D // 512)
                stats = statp.tile([P, nsub, 6], F32, tag="bnst")
                xv = xt.rearrange("p (s f) -> p s f", s=nsub)
                for s in range(nsub):
                    nc.vector.bn_stats(out=stats[:tsz, s, :], in_=xv[:tsz, s, :])
                nc.vector.bn_aggr(out=mvs[:tsz, ti, :], in_=stats[:tsz])
            istd = statp.tile([P, NTT], F32, tag="istd" + tag)
            nc.scalar.activation(out=istd[:, :], in_=mvs[:, :, 1],
                                 func=AF.Sqrt, bias=eps_t[:])
            nc.vector.reciprocal(out=istd[:, :], in_=istd[:, :])
            xns = []
            for ti, (xt, t0, tsz) in enumerate(xts):
                xn = pool.tile([P, D], BF, tag=tag)
                nc.vector.tensor_scalar(
                    out=xn[:tsz, :], in0=xt[:tsz, :],
                    scalar1=mvs[:tsz, ti, 0:1], scalar2=istd[:tsz, ti:ti + 1],
                    op0=ALU.subtract, op1=ALU.mult,
                )
                xns.append(xn)
            return xns

        def ln_transposes(xts, xns, dstT):
            """PE transposes of normalized tiles into feature-major dstT."""
            for ti, (xt, t0, tsz) in enumerate(xts):
                xn = xns[ti]
                trp = tr_psum()
                for kt in range(KD):
                    nc.tensor.matmul(
                        trp[:, kt, :tsz], xn[:tsz, kt * P:(kt + 1) * P],
                        identA[:tsz, :tsz], is_transpose=True,
                        start=(kt == 0), stop=(kt == KD - 1),
                    )
                nc.vector.tensor_copy(
                    out=dstT[:, :, t0:t0 + tsz], in_=trp[:, :, :tsz])

        def layernorm_transpose(xts, dstT, pool, tag):
            ln_transposes(xts, ln_normalize(xts, pool, tag), dstT)

        def phase_A_stats(g):
            """DMA x tiles + LN1 stats/normalize (no PE work), x += proj_b."""
            xn1T = xn1T_p.tile([P, KD, T], BF)
            xts = []
            for (t0, tsz) in t_tiles:
                xt = xpool.tile([P, D], F32)
                nc.sync.dma_start(out=xt[:tsz, :],
                                  in_=x_d[g * T + t0:g * T + t0 + tsz, :])
                xts.append((xt, t0, tsz))
            xns = ln_normalize(xts, xn_p, "xn1")
            for (xt, t0, tsz) in xts:
                nc.vector.tensor_tensor(out=xt[:tsz, :], in0=xt[:tsz, :],
                                        in1=pb_rep[:tsz, :], op=ALU.add)
            return xts, xns, xn1T

        state = phase_A_stats(0)
        for g in range(B):
            xts, xns, xn1T = state
            ln_transposes(xts, xns, xn1T)

            # ---------------- Phase B: QKV + attention ----------------
            aT = aT_p.tile([P, NPAIR, T], BF)
            pend = []
            avq = []

            def av_gen(entry):
                """Generator emitting the AV matmuls + reciprocal chain for a
                head whose scores/exp were emitted one head earlier. Yields
                after each matmul so the caller can weave these between score
                blocks (AV uses psAV, scores use psA — they interleave freely
                on the in-order PE queue)."""
                p_, h_, expT_, vkd_ = entry
                hb0_ = 64 * h_
                for (q0, qsz) in TB:
                    av = psAV.tile([P, 512], F32, tag="av")
                    for kti, (k0, ksz) in enumerate(t_tiles):
                        nc.tensor.matmul(
                            av[0:65, :qsz],
                            vkd_[:ksz, kti, h_, :],
                            expT_[:ksz, kti, q0:q0 + qsz],
                            start=(kti == 0), stop=(kti == NTT - 1),
                        )
                        yield
                    srow = rowp.tile([1, 512], F32, tag="srow")
                    nc.vector.reciprocal(out=srow[0:1, :qsz],
                                         in_=av[64:65, :qsz])
                    sinv = rowp.tile([1, 512], BF, tag="sinv")
                    nc.vector.tensor_copy(out=sinv[0:1, :qsz],
                                          in_=srow[0:1, :qsz])
                    pend.append((av, sinv, hb0_, p_, q0, qsz))

            def norm_flush():
                """Deferred per-head normalization: 1/denom broadcast matmul +
                DVE multiply into aT. Emitted one head late so the reciprocal
                chain latency hides behind the next head's scores."""
                for (av_, sinv_, hb0_, p_, q0_, qsz_) in pend:
                    bc = psA.tile([P, 512], F32, tag="s")
                    nc.tensor.matmul(bc[0:64, :qsz_], ones1[0:1, :],
                                     sinv_[0:1, 0:qsz_])
                    # DVE ops may read at most one PSUM operand: stage the
                    # broadcast to SBUF, then multiply against the PSUM av.
                    bcs = bcs_p.tile([P, 512], BF, tag="bcs")
                    nc.vector.tensor_copy(out=bcs[0:64, :qsz_],
                                          in_=bc[0:64, :qsz_])
                    nc.vector.tensor_tensor(
                        out=aT[hb0_:hb0_ + 64, p_, q0_:q0_ + qsz_],
                        in0=av_[0:64, :qsz_],
                        in1=bcs[0:64, :qsz_], op=ALU.mult)
                pend.clear()

            for p in range(NPAIR):
                wts = []
                for kt in range(KD):
                    wt = wq_p.tile([P, 3 * P], BF)
                    nc.sync.dma_start(out=wt[:, :],
                                      in_=wqkv_d[kt * P:(kt + 1) * P, p, :])
                    wts.append(wt)
                qT = qkvt_p.tile([P, T], BF, tag="tq")
                kT = qkvt_p.tile([P, T], BF, tag="tk")
                vT = qkvt_p.tile([P, T], BF, tag="tv")

                def qkv_mm(i, dst, on_act=False):
                    for (n0, nsz) in TB:
                        ps = psA.tile([P, 512], F32, tag="s")
                        for kt in range(KD):
                            nc.tensor.matmul(
                                ps[:, :nsz], wts[kt][:, i * P:(i + 1) * P],
                                xn1T[:, kt, n0:n0 + nsz],
                                start=(kt == 0), stop=(kt == KD - 1),
                            )
                        bias = cq_sb[:, p * 3 + i:p * 3 + i + 1]
                        if on_act:
                            # Identity is filler in every ACT table set: no
                            # table reload even interleaved with Exp/Gelu.
                            nc.scalar.add(dst[:, n0:n0 + nsz], ps[:, :nsz],
                                          bias)
                        else:
                            nc.vector.tensor_scalar_add(
                                out=dst[:, n0:n0 + nsz], in0=ps[:, :nsz],
                                scalar1=bias)

                qkv_mm(1, kT, on_act=True)
                qkv_mm(0, qT)
                vkd = vkd_p.tile([P, NTT, 2, 65], BF)
                nc.vector.memset(vkd[:, :, :, 64:65], 1.0)

                def v_and_transpose():
                    qkv_mm(2, vT)
                    for kti, (k0, ksz) in enumerate(t_tiles):
                        trp = tr_psum()
                        nc.tensor.matmul(trp[:ksz, 0, :], vT[:, k0:k0 + ksz],
                                         identA[:, :], is_transpose=True)
                        for hh in range(2):
                            nc.vector.tensor_copy(
                                out=vkd[:ksz, kti, hh, 0:64],
                                in_=trp[:ksz, 0, hh * 64:(hh + 1) * 64])

                for h in range(2):
                    hb0 = 64 * h
                    expT = expT_p.tile([P, NTT, T], BF)
                    norm_flush()
                    gen = av_gen(avq.pop()) if avq else iter(())
                    for kti, (k0, ksz) in enumerate(t_tiles):
                        for (q0, qsz) in TB:
                            sp = psA.tile([P, 512], F32, tag="s")
                            nc.tensor.matmul(
                                sp[:ksz, :qsz],
                                kT[hb0:hb0 + 64, k0:k0 + ksz],
                                qT[hb0:hb0 + 64, q0:q0 + qsz],
                            )
                            nc.scalar.activation(
                                out=expT[:ksz, kti, q0:q0 + qsz],
                                in_=sp[:ksz, :qsz], func=AF.Exp, scale=scale)
                        next(gen, None)
                        next(gen, None)
                    for _ in gen:
                        pass
                    avq.append((p, h, expT, vkd))
                    if h == 0:
                        # V for this pair lands between the two heads' scores,
                        # giving DVE time to evict k/q and ACT time for exp.
                        v_and_transpose()

            norm_flush()
            for _ in av_gen(avq.pop()):
                pass
            norm_flush()

            # ---------------- Phase C: proj + residual + LN2 ----------------
            wps = []
            for kt in range(KD):
                wt = w5_p.tile([P, D], BF, tag="w")
                nc.sync.dma_start(out=wt[:, :], in_=wp_d[kt * P:(kt + 1) * P, :])
                wps.append(wt)
            for tt, (t0, tsz) in enumerate(t_tiles):
                xt = xts[tt][0]
                for (o0, osz) in d_blocks:
                    ps = psM.tile([P, 512], F32, tag="mm")
                    for kt in range(KD):
                        nc.tensor.matmul(
                            ps[:tsz, :osz], aT[:, kt, t0:t0 + tsz],
                            wps[kt][:, o0:o0 + osz],
                            start=(kt == 0), stop=(kt == KD - 1),
                        )
                    # x2 = proj + (x + proj_b), in place
                    nc.vector.tensor_tensor(out=xt[:tsz, o0:o0 + osz],
                                            in0=ps[:tsz, :osz],
                                            in1=xt[:tsz, o0:o0 + osz],
                                            op=ALU.add)
            xn2T = xn2T_p.tile([P, KD, T], BF)
            layernorm_transpose(xts, xn2T, xn2_p, "xn2")
            # output accumulator = x2 + fc2_bias
            oacc = oacc_p.tile([P, NTT, D], F32)
            for tt, (t0, tsz) in enumerate(t_tiles):
                xt = xts[tt][0]
                nc.vector.tensor_tensor(out=oacc[:tsz, tt, :], in0=xt[:tsz, :],
                                        in1=cf2_rep[:tsz, :], op=ALU.add)

            # prefetch + LN-stats for next group before the MLP weight flood;
            # its PE transposes are emitted at the top of the next iteration.
            if g + 1 < B:
                state = phase_A_stats(g + 1)

            # ---------------- Phase D: MLP in 1024-wide hidden blocks ----------------
            for hb in range(NHB):
                f1s = []
                for kt in range(KD):
                    wt = w5_p.tile([P, HB], BF, tag="w")
                    nc.sync.dma_start(
                        out=wt[:, :],
                        in_=wf1_d[kt * P:(kt + 1) * P, hb * HB:(hb + 1) * HB])
                    f1s.append(wt)
                hT = hT_p.tile([P, KH, T], BF)
                for ht in range(KH):
                    for (n0, nsz) in TB:
                        ps = psM.tile([P, 512], F32, tag="mm")
                        for kt in range(KD):
                            nc.tensor.matmul(
                                ps[:, :nsz], f1s[kt][:, ht * P:(ht + 1) * P],
                                xn2T[:, kt, n0:n0 + nsz],
                                start=(kt == 0), stop=(kt == KD - 1),
                            )
                        nc.scalar.activation(
                            out=hT[:, ht, n0:n0 + nsz], in_=ps[:, :nsz],
                            func=_ACT_MLP,
                            bias=cf1_sb[:, hb * KH + ht:hb * KH + ht + 1])
                f2s = []
                for kt in range(KH):
                    wt = w5_p.tile([P, D], BF, tag="w")
                    nc.sync.dma_start(
                        out=wt[:, :],
                        in_=wf2_d[(hb * KH + kt) * P:(hb * KH + kt + 1) * P, :])
                    f2s.append(wt)
                for tt, (t0, tsz) in enumerate(t_tiles):
                    for (o0, osz) in d_blocks:
                        ps = psM.tile([P, 512], F32, tag="mm")
                        for kt in range(KH):
                            nc.tensor.matmul(
                                ps[:tsz, :osz], hT[:, kt, t0:t0 + tsz],
                                f2s[kt][:, o0:o0 + osz],
                                start=(kt == 0), stop=(kt == KH - 1),
                            )
                        nc.vector.tensor_tensor(
                            out=oacc[:tsz, tt, o0:o0 + osz],
                            in0=oacc[:tsz, tt, o0:o0 + osz],
                            in1=ps[:tsz, :osz], op=ALU.add)
            for tt, (t0, tsz) in enumerate(t_tiles):
                nc.sync.dma_start(out=out_d[g * T + t0:g * T + t0 + tsz, :],
                                  in_=oacc[:tsz, tt, :])

    nc.compile()
    return nc


def prepare_inputs(inputs, B, T, D, H, HID, n_cores):
    """Host-side folding/permutation. Returns per-core in_maps."""
    import ml_dtypes
    bf16 = ml_dtypes.bfloat16
    f8 = np.float64
    x = np.asarray(inputs["x"], np.float32)
    g1 = np.asarray(inputs["ln1_g"], f8)
    b1 = np.asarray(inputs["ln1_b"], f8)
    qkv_w = np.asarray(inputs["qkv_w"], f8)
    qkv_b = np.asarray(inputs["qkv_b"], f8)
    proj_w = np.asarray(inputs["proj_w"], np.float32)
    proj_b = np.asarray(inputs["proj_b"], np.float32)
    g2 = np.asarray(inputs["ln2_g"], f8)
    b2 = np.asarray(inputs["ln2_b"], f8)
    fc1_w = np.asarray(inputs["fc1_w"], f8)
    fc1_b = np.asarray(inputs["fc1_b"], f8)
    fc2_w = np.asarray(inputs["fc2_w"], np.float32)
    fc2_b = np.asarray(inputs["fc2_b"], np.float32)

    NPAIR = H // 2
    NHT = HID // P

    wq = (g1[:, None] * qkv_w).astype(np.float32)          # LN1 gamma folded
    cq = (b1 @ qkv_w + qkv_b).astype(np.float32)           # LN1 beta + qkv bias
    wq_, wk_, wv_ = wq[:, :D], wq[:, D:2 * D], wq[:, 2 * D:]
    wqkv = np.concatenate([
        wq_.reshape(D, NPAIR, P), wk_.reshape(D, NPAIR, P), wv_.reshape(D, NPAIR, P)
    ], axis=2).astype(bf16)
    cq_, ck_, cv_ = cq[:D], cq[D:2 * D], cq[2 * D:]
    cq_t = np.stack([cq_.reshape(NPAIR, P), ck_.reshape(NPAIR, P),
                     cv_.reshape(NPAIR, P)], axis=1)       # [NPAIR, 3, P]
    cq_t = np.ascontiguousarray(cq_t.transpose(2, 0, 1).reshape(P, NPAIR * 3),
                                dtype=np.float32)

    wf1 = (g2[:, None] * fc1_w).astype(bf16)
    cf1 = (b2 @ fc1_w + fc1_b).astype(np.float32)
    cf1_t = np.ascontiguousarray(cf1.reshape(NHT, P).T, dtype=np.float32)

    Bc = B // n_cores
    TOK = Bc * T
    shared = dict(wqkv=wqkv, cq=cq_t, wproj=proj_w.astype(bf16),
                  pb=proj_b.astype(np.float32),
                  wfc1=wf1, cf1=cf1_t,
                  wfc2=fc2_w.astype(bf16), cf2=fc2_b.astype(np.float32))
    in_maps = []
    for c in range(n_cores):
        m = dict(shared)
        m["x"] = np.ascontiguousarray(x[c * Bc:(c + 1) * Bc].reshape(TOK, D))
        in_maps.append(m)
    return in_maps


_NC_CACHE = {}


def _get_nc(B, T, D, H, HD, HID):
    key = (B, T, D, H, HD, HID)
    if key not in _NC_CACHE:
        _NC_CACHE[key] = build_nc(B=B, T=T, D=D, H=H, HD=HD, HID=HID)
    return _NC_CACHE[key]


def _run(inputs, trace=False):
    from concourse.bass_utils import run_bass_kernel_spmd
    x = np.asarray(inputs["x"])
    B, T, D = x.shape
    H = 16
    HD = D // H
    HID = np.asarray(inputs["fc1_w"]).shape[1]
    n_cores = 8
    Bc = B // n_cores
    nc = _get_nc(Bc, T, D, H, HD, HID)
    in_maps = prepare_inputs(inputs, B, T, D, H, HID, n_cores)
    res = run_bass_kernel_spmd(nc, in_maps, list(range(n_cores)), trace=trace)
    out = np.concatenate(
        [res.results[c]["out"].reshape(Bc, T, D) for c in range(n_cores)], axis=0)
    return out, res


def kernel(**inputs) -> np.ndarray:
    out, _ = _run(inputs, trace=False)
    return out.astype(np.float32)


# revision 24
# speedup vs baseline: 91.5776x; 91.5776x over previous
"""Fused ViT transformer block on Trainium2, data-parallel over batch across 8 cores.

v2: all-bf16 matmuls (fp32 PSUM accumulate), S^T-layout attention.

Per-core kernel (batch shard [4, 577, 1024]) computes the whole block:
  x + Attn(LN1(x)) -> x2 ; x2 + MLP(LN2(x2))
with all activations resident in SBUF.

Key layout choices:
  - LayerNorms run token-major [t, D] (bn_stats free-dim reductions), write a
    bf16 normalized copy, PE-transpose it into feature-major xnT [D, t] that
    feeds every matmul with the contraction dim on partitions. The raw x tiles
    stay resident in fp32 and serve as the residual stream (proj_b added
    on-chip via a broadcast tile).
  - Attention per head computes S^T[k, q] directly (lhsT = kT k-tile, rhs =
    qT), exp on ACT into bf16 expT, and AV as [V | 1]^T @ expT: the appended
    ones column makes PSUM row 64 the softmax denominators for free. A K=1
    ones-matmul broadcasts 1/denom across partitions and a single DVE multiply
    writes the normalized head output into aT (partition-shifted write for odd
    heads). No post-softmax transpose at all.
  - proj/fc2 are token-major (tokens on PSUM partitions, weights moving), fc1
    feature-major (tokens moving); MLP runs in 1024-wide hidden blocks with an
    fp32 SBUF accumulator initialized to x2 + fc2_bias.

Host-side precompute folds LN affine params into the adjacent weights
(w' = g[:,None]*w, c = b@w + bias); weights are shipped bf16.
"""

import numpy as np
from contextlib import ExitStack

import concourse.bass as bass
import concourse.mybir as mybir
import concourse.tile as tile
from concourse import bacc
from concourse.masks import make_identity

P = 128
F32 = mybir.dt.float32
BF = mybir.dt.bfloat16
AF = mybir.ActivationFunctionType
ALU = mybir.AluOpType


def _chunks(total, step):
    return [(i, min(step, total - i)) for i in range(0, total, step)]


# CoreSim lacks a Gelu implementation; vtest.py overrides this to Tanh.
_ACT_MLP = AF.Gelu


def build_nc(B=4, T=577, D=1024, H=16, HD=64, HID=4096, eps=1e-6, reps=1):
    """Build the single-core Bass program for a [B, T, D] shard."""
    assert D % P == 0 and H % 2 == 0 and HD == 64 and HID % 1024 == 0
    KD = D // P              # k-tiles over model dim (8)
    NPAIR = H // 2           # head pairs (8)
    NHT = HID // P           # fc1 output tiles (32)
    HB = 1024                # hidden block
    NHB = HID // HB          # 4
    KH = HB // P             # 8
    TOK = B * T
    scale = 1.0 / float(np.sqrt(HD))

    t_tiles = _chunks(T, P)          # [(0,128)x4, (512,65)] token tiles
    NTT = len(t_tiles)
    TB = _chunks(T, 512)             # token col blocks (N axis)
    d_blocks = _chunks(D, 512)

    nc = bacc.Bacc(None, target_bir_lowering=False, debug=False)

    x_d = nc.dram_tensor("x", [TOK, D], F32, kind="ExternalInput")
    wqkv_d = nc.dram_tensor("wqkv", [D, NPAIR, 3 * P], BF, kind="ExternalInput")
    cq_d = nc.dram_tensor("cq", [P, NPAIR * 3], F32, kind="ExternalInput")
    wp_d = nc.dram_tensor("wproj", [D, D], BF, kind="ExternalInput")
    pb_d = nc.dram_tensor("pb", [D], F32, kind="ExternalInput")
    wf1_d = nc.dram_tensor("wfc1", [D, HID], BF, kind="ExternalInput")
    cf1_d = nc.dram_tensor("cf1", [P, NHT], F32, kind="ExternalInput")
    wf2_d = nc.dram_tensor("wfc2", [HID, D], BF, kind="ExternalInput")
    cf2_d = nc.dram_tensor("cf2", [D], F32, kind="ExternalInput")
    out_d = nc.dram_tensor("out", [TOK, D], F32, kind="ExternalOutput")

    with tile.TileContext(nc) as tc, ExitStack() as ctx:
        const = ctx.enter_context(tc.tile_pool(name="const", bufs=1))
        xpool = ctx.enter_context(tc.tile_pool(name="xin", bufs=7))
        xn_p = ctx.enter_context(tc.tile_pool(name="xn", bufs=5))
        xn2_p = ctx.enter_context(tc.tile_pool(name="xn2", bufs=2))
        statp = ctx.enter_context(tc.tile_pool(name="stat", bufs=8))
        rowp = ctx.enter_context(tc.tile_pool(name="rows", bufs=5))
        bcs_p = ctx.enter_context(tc.tile_pool(name="bcs", bufs=2))
        xn1T_p = ctx.enter_context(tc.tile_pool(name="xn1T", bufs=2))
        wq_p = ctx.enter_context(tc.tile_pool(name="wq", bufs=12))
        qkvt_p = ctx.enter_context(tc.tile_pool(name="qkvt", bufs=2))
        vkd_p = ctx.enter_context(tc.tile_pool(name="vkd", bufs=2))
        expT_p = ctx.enter_context(tc.tile_pool(name="expT", bufs=2))
        aT_p = ctx.enter_context(tc.tile_pool(name="aT", bufs=1))
        w5_p = ctx.enter_context(tc.tile_pool(name="w5", bufs=10))
        xn2T_p = ctx.enter_context(tc.tile_pool(name="xn2T", bufs=2))
        hT_p = ctx.enter_context(tc.tile_pool(name="hT", bufs=2))
        oacc_p = ctx.enter_context(tc.tile_pool(name="oacc", bufs=1))
        psA = ctx.enter_context(tc.tile_pool(name="psA", bufs=3, space="PSUM"))
        psAV = ctx.enter_context(tc.tile_pool(name="psAV", bufs=2, space="PSUM"))
        psM = ctx.enter_context(tc.tile_pool(name="psM", bufs=3, space="PSUM"))

        def tr_psum():
            """bf16 [P, 8, 128] transpose target aliased onto an s-ring tile."""
            t = psA.tile([P, 512], F32, tag="s")
            return t.bitcast(BF).rearrange("p (k f) -> p k f", k=8)

        ident = const.tile([P, P], F32)
        make_identity(nc, ident)
        identA = const.tile([P, P], BF)
        nc.vector.tensor_copy(out=identA[:, :], in_=ident[:, :])
        eps_t = const.tile([P, 1], F32)
        nc.vector.memset(eps_t, eps)
        ones1 = const.tile([1, 64], BF)
        nc.vector.memset(ones1, 1.0)
        cq_sb = const.tile([P, NPAIR * 3], F32)
        nc.sync.dma_start(out=cq_sb[:, :], in_=cq_d[:, :])
        cf1_sb = const.tile([P, NHT], F32)
        nc.sync.dma_start(out=cf1_sb[:, :], in_=cf1_d[:, :])

        def bcast_row(dram_vec, tag):
            t = const.tile([P, D], F32, tag=tag)
            ap = dram_vec[:]
            nc.sync.dma_start(
                out=t[:, :],
                in_=bass.AP(tensor=ap.tensor, offset=ap.offset,
                            ap=[[0, P]] + list(ap.ap)),
            )
            return t

        pb_rep = bcast_row(pb_d, "pbrep")
        cf2_rep = bcast_row(cf2_d, "cf2rep")

        def ln_normalize(xts, pool, tag):
            """bn_stats/aggr + batched sqrt/recip + normalized bf16 copies.
            DVE/ACT only — no PE instructions."""
            mvs = statp.tile([P, NTT, 2], F32, tag="mv" + tag)
            nc.vector.memset(mvs, 1.0)
            for ti, (xt, t0, tsz) in enumerate(xts):
                nsub = max(1, D // 512)
                stats = statp.tile([P, nsub, 6], F32, tag="bnst")
                xv = xt.rearrange("p (s f) -> p s f", s=nsub)
                for s in range(nsub):
                    nc.vector.bn_stats(out=stats[:tsz, s, :], in_=xv[:tsz, s, :])
                nc.vector.bn_aggr(out=mvs[:tsz, ti, :], in_=stats[:tsz])
            istd = statp.tile([P, NTT], F32, tag="istd" + tag)
            nc.scalar.activation(out=istd[:, :], in_=mvs[:, :, 1],
                                 func=AF.Sqrt, bias=eps_t[:])
            nc.vector.reciprocal(out=istd[:, :], in_=istd[:, :])
            xns = []
            for ti, (xt, t0, tsz) in enumerate(xts):
                xn = pool.tile([P, D], BF, tag=tag)
                nc.vector.tensor_scalar(
                    out=xn[:tsz, :], in0=xt[:tsz, :],
                    scalar1=mvs[:tsz, ti, 0:1], scalar2=istd[:tsz, ti:ti + 1],
                    op0=ALU.subtract, op1=ALU.mult,
                )
                xns.append(xn)
            return xns

        def ln_transposes(xts, xns, dstT):
            """PE transposes of normalized tiles into feature-major dstT."""
            for ti, (xt, t0, tsz) in enumerate(xts):
                xn = xns[ti]
                trp = tr_psum()
                for kt in range(KD):
                    nc.tensor.matmul(
                        trp[:, kt, :tsz], xn[:tsz, kt * P:(kt + 1) * P],
                        identA[:tsz, :tsz], is_transpose=True,
                        start=(kt == 0), stop=(kt == KD - 1),
                    )
                nc.vector.tensor_copy(
                    out=dstT[:, :, t0:t0 + tsz], in_=trp[:, :, :tsz])

        def layernorm_transpose(xts, dstT, pool, tag):
            ln_transposes(xts, ln_normalize(xts, pool, tag), dstT)

        def phase_A_stats(g):
            """DMA x tiles + LN1 stats/normalize (no PE work), x += proj_b."""
            xn1T = xn1T_p.tile([P, KD, T], BF)
            xts = []
            for (t0, tsz) in t_tiles:
                xt = xpool.tile([P, D], F32)
                nc.sync.dma_start(out=xt[:tsz, :],
                                  in_=x_d[g * T + t0:g * T + t0 + tsz, :])
                xts.append((xt, t0, tsz))
            xns = ln_normalize(xts, xn_p, "xn1")
            for (xt, t0, tsz) in xts:
                nc.vector.tensor_tensor(out=xt[:tsz, :], in0=xt[:tsz, :],
                                        in1=pb_rep[:tsz, :], op=ALU.add)
            return xts, xns, xn1T

        state = phase_A_stats(0)
        NG = B * reps
        for gi in range(NG):
            g = gi % B
            xts, xns, xn1T = state
            ln_transposes(xts, xns, xn1T)

            # ---------------- Phase B: QKV + attention ----------------
            aT = aT_p.tile([P, NPAIR, T], BF)
            pend = []
            avq = []

            def av_gen(entry):
                """Generator emitting the AV matmuls + reciprocal chain for a
                head whose scores/exp were emitted one head earlier. Yields
                after each matmul so the caller can weave these between score
                blocks (AV uses psAV, scores use psA — they interleave freely
                on the in-order PE queue)."""
                p_, h_, expT_, vkd_ = entry
                hb0_ = 64 * h_
                for (q0, qsz) in TB:
                    av = psAV.tile([P, 512], F32, tag="av")
                    for kti, (k0, ksz) in enumerate(t_tiles):
                        nc.tensor.matmul(
                            av[0:65, :qsz],
                            vkd_[:ksz, kti, h_, :],
                            expT_[:ksz, kti, q0:q0 + qsz],
                            start=(kti == 0), stop=(kti == NTT - 1),
                        )
                        yield
                    srow = rowp.tile([1, 512], F32, tag="srow")
                    nc.vector.reciprocal(out=srow[0:1, :qsz],
                                         in_=av[64:65, :qsz])
                    sinv = rowp.tile([1, 512], BF, tag="sinv")
                    nc.vector.tensor_copy(out=sinv[0:1, :qsz],
                                          in_=srow[0:1, :qsz])
                    pend.append((av, sinv, hb0_, p_, q0, qsz))

            def norm_flush():
                """Deferred per-head normalization: 1/denom broadcast matmul +
                DVE multiply into aT. Emitted one head late so the reciprocal
                chain latency hides behind the next head's scores."""
                for (av_, sinv_, hb0_, p_, q0_, qsz_) in pend:
                    bc = psA.tile([P, 512], F32, tag="s")
                    nc.tensor.matmul(bc[0:64, :qsz_], ones1[0:1, :],
                                     sinv_[0:1, 0:qsz_])
                    # DVE ops may read at most one PSUM operand: stage the
                    # broadcast to SBUF, then multiply against the PSUM av.
                    bcs = bcs_p.tile([P, 512], BF, tag="bcs")
                    nc.vector.tensor_copy(out=bcs[0:64, :qsz_],
                                          in_=bc[0:64, :qsz_])
                    nc.vector.tensor_tensor(
                        out=aT[hb0_:hb0_ + 64, p_, q0_:q0_ + qsz_],
                        in0=av_[0:64, :qsz_],
                        in1=bcs[0:64, :qsz_], op=ALU.mult)
                pend.clear()

            for p in range(NPAIR):
                wts = []
                for kt in range(KD):
                    wt = wq_p.tile([P, 3 * P], BF)
                    nc.sync.dma_start(out=wt[:, :],
                                      in_=wqkv_d[kt * P:(kt + 1) * P, p, :])
                    wts.append(wt)
                qT = qkvt_p.tile([P, T], BF, tag="tq")
                kT = qkvt_p.tile([P, T], BF, tag="tk")
                vT = qkvt_p.tile([P, T], BF, tag="tv")

                def qkv_mm(i, dst, on_act=False):
                    for (n0, nsz) in TB:
                        ps = psA.tile([P, 512], F32, tag="s")
                        for kt in range(KD):
                            nc.tensor.matmul(
                                ps[:, :nsz], wts[kt][:, i * P:(i + 1) * P],
                                xn1T[:, kt, n0:n0 + nsz],
                                start=(kt == 0), stop=(kt == KD - 1),
                            )
                        bias = cq_sb[:, p * 3 + i:p * 3 + i + 1]
                        if on_act:
                            # Identity is filler in every ACT table set: no
                            # table reload even interleaved with Exp/Gelu.
                            nc.scalar.add(dst[:, n0:n0 + nsz], ps[:, :nsz],
                                          bias)
                        else:
                            nc.vector.tensor_scalar_add(
                                out=dst[:, n0:n0 + nsz], in0=ps[:, :nsz],
                                scalar1=bias)

                qkv_mm(1, kT, on_act=True)
                qkv_mm(0, qT)
                vkd = vkd_p.tile([P, NTT, 2, 65], BF)
                nc.vector.memset(vkd[:, :, :, 64:65], 1.0)

                def v_and_transpose():
                    qkv_mm(2, vT)
                    for kti, (k0, ksz) in enumerate(t_tiles):
                        trp = tr_psum()
                        nc.tensor.matmul(trp[:ksz, 0, :], vT[:, k0:k0 + ksz],
                                         identA[:, :], is_transpose=True)
                        for hh in range(2):
                            nc.vector.tensor_copy(
                                out=vkd[:ksz, kti, hh, 0:64],
                                in_=trp[:ksz, 0, hh * 64:(hh + 1) * 64])

                for h in range(2):
                    hb0 = 64 * h
                    expT = expT_p.tile([P, NTT, T], BF)
                    norm_flush()
                    gen = av_gen(avq.pop()) if avq else iter(())
                    for kti, (k0, ksz) in enumerate(t_tiles):
                        for (q0, qsz) in TB:
                            sp = psA.tile([P, 512], F32, tag="s")
                            nc.tensor.matmul(
                                sp[:ksz, :qsz],
                                kT[hb0:hb0 + 64, k0:k0 + ksz],
                                qT[hb0:hb0 + 64, q0:q0 + qsz],
                            )
                            nc.scalar.activation(
                                out=expT[:ksz, kti, q0:q0 + qsz],
                                in_=sp[:ksz, :qsz], func=AF.Exp, scale=scale)
                        next(gen, None)
                        next(gen, None)
                    for _ in gen:
                        pass
                    avq.append((p, h, expT, vkd))
                    if h == 0:
                        # V for this pair lands between the two heads' scores,
                        # giving DVE time to evict k/q and ACT time for exp.
                        v_and_transpose()

            norm_flush()
            for _ in av_gen(avq.pop()):
                pass
            norm_flush()

            # ---------------- Phase C: proj + residual + LN2 ----------------
            wps = []
            for kt in range(KD):
                wt = w5_p.tile([P, D], BF, tag="w")
                nc.sync.dma_start(out=wt[:, :], in_=wp_d[kt * P:(kt + 1) * P, :])
                wps.append(wt)
            for tt, (t0, tsz) in enumerate(t_tiles):
                xt = xts[tt][0]
                for (o0, osz) in d_blocks:
                    ps = psM.tile([P, 512], F32, tag="mm")
                    for kt in range(KD):
                        nc.tensor.matmul(
                            ps[:tsz, :osz], aT[:, kt, t0:t0 + tsz],
                            wps[kt][:, o0:o0 + osz],
                            start=(kt == 0), stop=(kt == KD - 1),
                        )
                    # x2 = proj + (x + proj_b), in place
                    nc.vector.tensor_tensor(out=xt[:tsz, o0:o0 + osz],
                                            in0=ps[:tsz, :osz],
                                            in1=xt[:tsz, o0:o0 + osz],
                                            op=ALU.add)
            xn2T = xn2T_p.tile([P, KD, T], BF)
            layernorm_transpose(xts, xn2T, xn2_p, "xn2")
            # output accumulator = x2 + fc2_bias
            oacc = oacc_p.tile([P, NTT, D], F32)
            for tt, (t0, tsz) in enumerate(t_tiles):
                xt = xts[tt][0]
                nc.vector.tensor_tensor(out=oacc[:tsz, tt, :], in0=xt[:tsz, :],
                                        in1=cf2_rep[:tsz, :], op=ALU.add)

            # prefetch + LN-stats for next group before the MLP weight flood;
            # its PE transposes are emitted at the top of the next iteration.
            if gi + 1 < NG:
                state = phase_A_stats((gi + 1) % B)

            # ---------------- Phase D: MLP in 1024-wide hidden blocks ----------------
            for hb in range(NHB):
                f1s = []
                for kt in range(KD):
                    wt = w5_p.tile([P, HB], BF, tag="w")
                    nc.sync.dma_start(
                        out=wt[:, :],
                        in_=wf1_d[kt * P:(kt + 1) * P, hb * HB:(hb + 1) * HB])
                    f1s.append(wt)
                hT = hT_p.tile([P, KH, T], BF)
                for ht in range(KH):
                    for (n0, nsz) in TB:
                        ps = psM.tile([P, 512], F32, tag="mm")
                        for kt in range(KD):
                            nc.tensor.matmul(
                                ps[:, :nsz], f1s[kt][:, ht * P:(ht + 1) * P],
                                xn2T[:, kt, n0:n0 + nsz],
                                start=(kt == 0), stop=(kt == KD - 1),
                            )
                        nc.scalar.activation(
                            out=hT[:, ht, n0:n0 + nsz], in_=ps[:, :nsz],
                            func=_ACT_MLP,
                            bias=cf1_sb[:, hb * KH + ht:hb * KH + ht + 1])
                f2s = []
                for kt in range(KH):
                    wt = w5_p.tile([P, D], BF, tag="w")
                    nc.sync.dma_start(
                        out=wt[:, :],
                        in_=wf2_d[(hb * KH + kt) * P:(hb * KH + kt + 1) * P, :])
                    f2s.append(wt)
                for tt, (t0, tsz) in enumerate(t_tiles):
                    for (o0, osz) in d_blocks:
                        ps = psM.tile([P, 512], F32, tag="mm")
                        for kt in range(KH):
                            nc.tensor.matmul(
                                ps[:tsz, :osz], hT[:, kt, t0:t0 + tsz],
                                f2s[kt][:, o0:o0 + osz],
                                start=(kt == 0), stop=(kt == KH - 1),
                            )
                        nc.vector.tensor_tensor(
                            out=oacc[:tsz, tt, o0:o0 + osz],
                            in0=oacc[:tsz, tt, o0:o0 + osz],
                            in1=ps[:tsz, :osz], op=ALU.add)
            for tt, (t0, tsz) in enumerate(t_tiles):
                nc.sync.dma_start(out=out_d[g * T + t0:g * T + t0 + tsz, :],
                                  in_=oacc[:tsz, tt, :])

    nc.compile()
    return nc


def prepare_inputs(inputs, B, T, D, H, HID, n_cores):
    """Host-side folding/permutation. Returns per-core in_maps."""
    import ml_dtypes
    bf16 = ml_dtypes.bfloat16
    f8 = np.float64
    x = np.asarray(inputs["x"], np.float32)
    g1 = np.asarray(inputs["ln1_g"], f8)
    b1 = np.asarray(inputs["ln1_b"], f8)
    qkv_w = np.asarray(inputs["qkv_w"], f8)
    qkv_b = np.asarray(inputs["qkv_b"], f8)
    proj_w = np.asarray(inputs["proj_w"], np.float32)
    proj_b = np.asarray(inputs["proj_b"], np.float32)
    g2 = np.asarray(inputs["ln2_g"], f8)
    b2 = np.asarray(inputs["ln2_b"], f8)
    fc1_w = np.asarray(inputs["fc1_w"], f8)
    fc1_b = np.asarray(inputs["fc1_b"], f8)
    fc2_w = np.asarray(inputs["fc2_w"], np.float32)
    fc2_b = np.asarray(inputs["fc2_b"], np.float32)

    NPAIR = H // 2
    NHT = HID // P

    wq = (g1[:, None] * qkv_w).astype(np.float32)          # LN1 gamma folded
    cq = (b1 @ qkv_w + qkv_b).astype(np.float32)           # LN1 beta + qkv bias
    wq_, wk_, wv_ = wq[:, :D], wq[:, D:2 * D], wq[:, 2 * D:]
    wqkv = np.concatenate([
        wq_.reshape(D, NPAIR, P), wk_.reshape(D, NPAIR, P), wv_.reshape(D, NPAIR, P)
    ], axis=2).astype(bf16)
    cq_, ck_, cv_ = cq[:D], cq[D:2 * D], cq[2 * D:]
    cq_t = np.stack([cq_.reshape(NPAIR, P), ck_.reshape(NPAIR, P),
                     cv_.reshape(NPAIR, P)], axis=1)       # [NPAIR, 3, P]
    cq_t = np.ascontiguousarray(cq_t.transpose(2, 0, 1).reshape(P, NPAIR * 3),
                                dtype=np.float32)

    wf1 = (g2[:, None] * fc1_w).astype(bf16)
    cf1 = (b2 @ fc1_w + fc1_b).astype(np.float32)
    cf1_t = np.ascontiguousarray(cf1.reshape(NHT, P).T, dtype=np.float32)

    Bc = B // n_cores
    TOK = Bc * T
    shared = dict(wqkv=wqkv, cq=cq_t, wproj=proj_w.astype(bf16),
                  pb=proj_b.astype(np.float32),
                  wfc1=wf1, cf1=cf1_t,
                  wfc2=fc2_w.astype(bf16), cf2=fc2_b.astype(np.float32))
    in_maps = []
    for c in range(n_cores):
        m = dict(shared)
        m["x"] = np.ascontiguousarray(x[c * Bc:(c + 1) * Bc].reshape(TOK, D))
        in_maps.append(m)
    return in_maps


_NC_CACHE = {}


def _get_nc(B, T, D, H, HD, HID):
    key = (B, T, D, H, HD, HID)
    if key not in _NC_CACHE:
        _NC_CACHE[key] = build_nc(B=B, T=T, D=D, H=H, HD=HD, HID=HID)
    return _NC_CACHE[key]


def _run(inputs, trace=False):
    from concourse.bass_utils import run_bass_kernel_spmd
    x = np.asarray(inputs["x"])
    B, T, D = x.shape
    H = 16
    HD = D // H
    HID = np.asarray(inputs["fc1_w"]).shape[1]
    n_cores = 8
    Bc = B // n_cores
    nc = _get_nc(Bc, T, D, H, HD, HID)
    in_maps = prepare_inputs(inputs, B, T, D, H, HID, n_cores)
    res = run_bass_kernel_spmd(nc, in_maps, list(range(n_cores)), trace=trace)
    out = np.concatenate(
        [res.results[c]["out"].reshape(Bc, T, D) for c in range(n_cores)], axis=0)
    return out, res


def kernel(**inputs) -> np.ndarray:
    out, _ = _run(inputs, trace=False)
    return out.astype(np.float32)
